# revision 45
# baseline (speedup 1.0000x reference)
"""Trainium2 Bass kernel for nn_Attention_22299470201527.

Dense transformer attention block:
  LayerNorm -> Wq/Wkv projections -> per-head QK RMSNorm -> 2D RoPE ->
  softmax(QK^T) V -> Wo projection,  B=8, N=1024, DIM=1024, H=16, DH=64.

Sharding: data-parallel over batch — 8 batch elements on 8 NeuronCores,
one per core, weights replicated, no collectives. kernel(**inputs) takes
the full unsharded inputs and returns the full [8, 1024, 1024] output.

Design notes (~335us HW exec, vs 397us for the f32r baseline):
  * All matmul operands fp16/bf16 (FWL weight loads, half the DMA bytes).
  * Weights/trig tables are relaid out host-side to partition-major so
    every DMA lands as 128 large contiguous descriptors (the naive
    layout cost ~27k small descriptors and ~30us of serialized startup).
  * x tiles are prefetched ahead of the weight streams; each weight
    matrix streams during the previous projection loop (wpool bufs=2).
  * LayerNorm is fused into the Q loop; LN math runs two iterations
    ahead and the RoPE-output PE transposes three behind, so the
    in-order PE queue never head-of-line blocks on the eviction ->
    square -> reduce -> rsqrt -> RoPE -> GpSimd-rinv chain (~9us).
  * QK^T (contraction K=DH=64) is issued as four [64-key, 512-query]
    quarter matmuls per head pair, arranged A-lo||B-hi / A-hi||B-lo so
    the two in-flight matmuls always have disjoint PE row groups AND
    disjoint PSUM output partitions: the hardware executes each pair
    concurrently as array tiles (observed: 3 of every 4 QK matmuls
    retire in <5ns), halving QK time.
  * V projection is interleaved with head-pair 0's QK/exp, and the
    first v-proj slots in before the final k transposes, so the PE
    stays dense across the B->C transition (a PE idle bubble there
    trips the HAM clock throttle, which takes tens of us to recover).
  * exp on ACT (~1.03us per [128,1024] tile) paces phase C; AV matmuls
    of pair f-1 are issued BEFORE QK of pair f each j-step (this order
    also makes es-pool recycling deadlock-free).
  * Softmax denominators via an all-ones 65th V column (memset, not
    DMA); per-pair normalization broadcasts reciprocals via a DRAM
    bounce and multiplies on the otherwise idle GpSimd engine. Raw AV
    output is evicted into dead qT storage (bf16 bitcast view) and the
    normalized fp16 result into the dead xnT buffer for Wo.
  * Wo accumulates head-pair blocks f=0..6 before f=7 so phase D
    starts while the last pair's normalization chain is in flight;
    output eviction alternates ACT/DVE and DMAs per 512-half.
"""

import sys

for _p in ("/opt/trn_rl_repo",):
    if _p not in sys.path:
        sys.path.append(_p)

import concourse.bacc as bacc
import concourse.bass as bass
import concourse.tile as tile
from concourse import mybir

F32 = mybir.dt.float32
F16 = mybir.dt.float16
BF16 = mybir.dt.bfloat16

B, N, DIM, H, DH = 8, 1024, 1024, 16, 64
INNER = H * DH
KT = DIM // 128
MT = N // 128
FT = INNER // 128
EPS_LN = 1e-5
EPS_NORM = 1e-12


def _bcast_heads(ap2d, nheads=H):
    """[128, D] AP -> [128, nheads, D], stride-0 broadcast over heads."""
    return bass.AP(
        tensor=ap2d.tensor, offset=ap2d.offset,
        ap=[ap2d.ap[0], [0, nheads], ap2d.ap[1]],
    )


def _bcast_last(ap2d, n):
    """[128, Hn] AP -> [128, Hn, n], stride-0 broadcast innermost."""
    return bass.AP(
        tensor=ap2d.tensor, offset=ap2d.offset,
        ap=[ap2d.ap[0], ap2d.ap[1], [0, n]],
    )


def _rot_view(tile_ap):
    """[128, 1024] tile viewed [128, H, 2, 2, 16] with adjacent 16-blocks
    swapped (rotate-half shuffle; signs live in the sin table)."""
    return bass.AP(
        tensor=tile_ap.tensor, offset=tile_ap.offset + 16,
        ap=[tile_ap.ap[0], [DH, H], [32, 2], [-16, 2], [1, 16]],
    )


def build_nc(has_bias: bool):
    nc = bacc.Bacc("TRN2", target_bir_lowering=False, debug=False, num_devices=8)

    x_d = nc.dram_tensor("x", [N, DIM], F32, kind="ExternalInput")
    # weights partition-major: [128, KT, INNER] flattened per partition
    wq_d = nc.dram_tensor("wq", [128, KT * INNER], F16, kind="ExternalInput")
    wk_d = nc.dram_tensor("wk", [128, KT * INNER], F16, kind="ExternalInput")
    wv_d = nc.dram_tensor("wv", [128, KT * INNER], F16, kind="ExternalInput")
    wo_d = nc.dram_tensor("wo", [128, KT * INNER], F16, kind="ExternalInput")
    id16_d = nc.dram_tensor("ident16", [128, 128], F16, kind="ExternalInput")
    cos_d = nc.dram_tensor("cos_t", [128, MT * DH], F16, kind="ExternalInput")
    sin_d = nc.dram_tensor("sin_t", [128, MT * DH], F16, kind="ExternalInput")
    if has_bias:
        bq_d = nc.dram_tensor("bq", [1, INNER], F16, kind="ExternalInput")
        bkv_d = nc.dram_tensor("bkv", [1, 2 * INNER], F16, kind="ExternalInput")
    out_d = nc.dram_tensor("out", [N, DIM], F32, kind="ExternalOutput")
    rd_dram = nc.dram_tensor("rd_scratch", [FT, 2, N], F32, kind="Internal")

    with tile.TileContext(nc) as tc:
        with (
            tc.tile_pool(name="const", bufs=1) as constp,
            tc.tile_pool(name="wpool", bufs=2) as wpool,
            tc.tile_pool(name="stats", bufs=2) as stats,
            tc.tile_pool(name="bc", bufs=1) as bc,
            tc.tile_pool(name="xa", bufs=1) as xa,
        ):
            # Prefetch first x tiles BEFORE everything else so LayerNorm
            # starts immediately.
            XPRE = 2
            x_tiles = {}

            def x_fetch(m):
                x_t = xa.tile([128, DIM], F32, tag="x", bufs=2, name=f"x{m%2}")
                nc.sync.dma_start(x_t[:], x_d[m * 128:(m + 1) * 128, :])
                x_tiles[m] = x_t

            for m in range(XPRE):
                x_fetch(m)

            ident_h = constp.tile([128, 128], F16)
            nc.sync.dma_start(ident_h[:], id16_d[:])
            eps_t = constp.tile([128, 1], F32)
            nc.vector.memset(eps_t[:], EPS_LN)

            def stream_w(dram_t):
                w = wpool.tile([128, KT, INNER], F16, tag="w")
                nc.sync.dma_start(
                    w[:], dram_t[:].rearrange("p (a i) -> p a i", a=KT)
                )
                return w

            w_q = stream_w(wq_d)

            cos_sb = constp.tile([128, MT, DH], F16)
            sin_sb = constp.tile([128, MT, DH], F16)
            nc.sync.dma_start(cos_sb[:], cos_d[:].rearrange("p (a d) -> p a d", a=MT))
            nc.sync.dma_start(sin_sb[:], sin_d[:].rearrange("p (a d) -> p a d", a=MT))
            bq_sb = bkv_sb = ones1 = None
            if has_bias:
                bq_sb = constp.tile([1, INNER], F16)
                bkv_sb = constp.tile([1, 2 * INNER], F16)
                nc.sync.dma_start(bq_sb[:], bq_d[:])
                nc.sync.dma_start(bkv_sb[:], bkv_d[:])
                ones1 = constp.tile([1, 128], F16)
                nc.vector.memset(ones1[:], 1.0)

            # Long-lived activations.
            qT = bc.tile([128, FT, N], F16)
            kT = bc.tile([128, FT, N], F16)
            xnT = bc.tile([128, KT, N], F16)   # reused as outT in phase C/D
            v_sb = bc.tile([128, MT, H, DH + 1], BF16)
            # ones column for the softmax denominator (DVE memset, not DMA:
            # a strided DMA here costs ~16k tiny descriptors)
            nc.vector.memset(
                bass.AP(
                    tensor=v_sb.tensor, offset=v_sb[:].offset + DH,
                    ap=[v_sb[:].ap[0], [H * (DH + 1), MT], [DH + 1, H]],
                ),
                1.0,
            )

            ep_cm = tc.tile_pool(name="ep", bufs=1)
            ep = ep_cm.__enter__()
            cp_cm = tc.tile_pool(name="cpool", bufs=1)
            cpool = cp_cm.__enter__()

            es_pair0 = {}
            with tc.tile_pool(name="pb", bufs=1) as pb:
                pp_cm = tc.tile_pool(name="pp", bufs=2, space="PSUM")
                pp = pp_cm.__enter__()
                tp_cm = tc.tile_pool(name="tp", bufs=4, space="PSUM")
                tp = tp_cm.__enter__()
                def ln_math(m):
                    """LayerNorm stats + normalized fp16 tile for x tile m."""
                    x_t = x_tiles.pop(m)
                    st = stats.tile([128, 2, 6], F32, tag="bst")
                    for g in range(2):
                        nc.vector.bn_stats(st[:, g, :], x_t[:, g * 512:(g + 1) * 512])
                    mv = stats.tile([128, 2], F32, tag="mv")
                    nc.vector.bn_aggr(mv[:], st[:])
                    sd = stats.tile([128, 1], F32, tag="sd")
                    nc.scalar.activation(
                        sd[:], mv[:, 1:2], mybir.ActivationFunctionType.Sqrt,
                        bias=eps_t[:], scale=1.0,
                    )
                    rstd = stats.tile([128, 1], F32, tag="rstd")
                    nc.vector.reciprocal(rstd[:], sd[:])
                    nmu = stats.tile([128, 1], F32, tag="nmu")
                    nc.vector.scalar_tensor_tensor(
                        out=nmu[:], in0=mv[:, 0:1], scalar=-1.0, in1=rstd[:],
                        op0=mybir.AluOpType.mult, op1=mybir.AluOpType.mult,
                    )
                    xn_t = xa.tile([128, DIM], F16, tag="xn", bufs=3)
                    nc.scalar.activation(
                        xn_t[:], x_t[:], mybir.ActivationFunctionType.Identity,
                        bias=nmu[:], scale=rstd[:],
                    )
                    if m + XPRE < MT:
                        x_fetch(m + XPRE)
                    return xn_t

                def ln_transp(xn_t, m):
                    for g in range(2):
                        tps = tp.tile([128, 512], F16, tag="tp", bufs=4)
                        for b4 in range(4):
                            k = g * 4 + b4
                            nc.tensor.transpose(
                                tps[:, b4 * 128:(b4 + 1) * 128],
                                xn_t[:, k * 128:(k + 1) * 128],
                                ident_h[:],
                            )
                        # split the two evictions across ACT and DVE so the
                        # dependent proj(m) isn't gated by the ACT queue
                        dst = xnT[:, g * 4:(g + 1) * 4, m * 128:(m + 1) * 128]
                        src = tps[:].rearrange("p (a t) -> p a t", a=4)
                        if g == 0:
                            nc.scalar.copy(dst, src)
                        else:
                            nc.vector.tensor_copy(dst, src)

                def proj(w, m, psp, bias_sb=None, bias_off=0):
                    """One [128, INNER] projection psum tile for token tile m.
                    Matmul moving free dim is capped at 512, so each half is
                    its own accumulation group."""
                    ps = psp.tile([128, INNER], F32, tag="pp", bufs=2)
                    for nh in range(2):
                        sl = slice(nh * 512, (nh + 1) * 512)
                        if bias_sb is not None:
                            nc.tensor.matmul(
                                ps[:, sl], ones1[:],
                                bias_sb[:, bias_off + nh * 512:
                                        bias_off + (nh + 1) * 512],
                                start=True, stop=False,
                            )
                        for k in range(KT):
                            nc.tensor.matmul(
                                ps[:, sl],
                                xnT[:, k, m * 128:(m + 1) * 128],
                                w[:, k, nh * 512:(nh + 1) * 512],
                                start=(k == 0 and bias_sb is None),
                                stop=(k == KT - 1),
                            )
                    return ps

                def rms_rope(ps, m):
                    qtmp = pb.tile([128, INNER], F16, tag="qtmp", bufs=2)
                    nc.scalar.copy(qtmp[:], ps[:])
                    sq = pb.tile([128, INNER], F16, tag="sq", bufs=1)
                    nc.scalar.activation(
                        sq[:], qtmp[:], mybir.ActivationFunctionType.Square,
                        bias=0.0, scale=1.0,
                    )
                    ssq = stats.tile([128, H], F32, tag="ssq")
                    nc.vector.reduce_sum(
                        ssq[:], sq[:].rearrange("p (h d) -> p h d", h=H),
                        axis=mybir.AxisListType.X,
                    )
                    nrm = stats.tile([128, H], F32, tag="nrm")
                    nc.scalar.activation(
                        nrm[:], ssq[:], mybir.ActivationFunctionType.Sqrt,
                        bias=0.0, scale=1.0,
                    )
                    nc.vector.tensor_scalar_max(nrm[:], nrm[:], EPS_NORM)
                    rinv = stats.tile([128, H], F32, tag="rinv")
                    nc.vector.reciprocal(rinv[:], nrm[:])

                    q3 = qtmp[:].rearrange("p (h d) -> p h d", h=H)
                    t1 = pb.tile([128, INNER], F16, tag="t1", bufs=2)
                    nc.vector.tensor_mul(
                        t1[:].rearrange("p (h d) -> p h d", h=H),
                        q3, _bcast_heads(cos_sb[:, m, :]),
                    )
                    t2 = pb.tile([128, INNER], F16, tag="t2", bufs=2)
                    sin_b = bass.AP(
                        tensor=sin_sb.tensor,
                        offset=sin_sb[:, m, :].offset,
                        ap=[sin_sb[:, m, :].ap[0], [0, H], [32, 2], [16, 2],
                            [1, 16]],
                    )
                    nc.vector.tensor_mul(
                        t2[:].rearrange("p (h a b c) -> p h a b c",
                                        h=H, a=2, b=2, c=16),
                        _rot_view(qtmp[:]), sin_b,
                    )
                    nc.gpsimd.tensor_add(t1[:], t1[:], t2[:])
                    qr = pb.tile([128, INNER], F16, tag="qr", bufs=4)
                    nc.gpsimd.tensor_mul(
                        qr[:].rearrange("p (h d) -> p h d", h=H),
                        t1[:].rearrange("p (h d) -> p h d", h=H),
                        _bcast_last(rinv[:], DH),
                    )
                    return qr

                def transp(qr, m, dst):
                    for g in range(2):
                        tps = tp.tile([128, 512], F16, tag="tp", bufs=4)
                        for b4 in range(4):
                            f = g * 4 + b4
                            nc.tensor.transpose(
                                tps[:, b4 * 128:(b4 + 1) * 128],
                                qr[:, f * 128:(f + 1) * 128],
                                ident_h[:],
                            )
                        nc.scalar.copy(
                            dst[:, g * 4:(g + 1) * 4, m * 128:(m + 1) * 128],
                            tps[:].rearrange("p (a t) -> p a t", a=4),
                        )

                # ---- Q loop (LayerNorm fused; LN math runs two iterations
                # ahead and RoPE-output transposes two behind so the PE
                # in-order queue never waits on the ACT/DVE/GPS chains) ----
                xns = {0: ln_math(0), 1: ln_math(1)}
                qrs = {}
                w_k = None
                for m in range(MT):
                    ln_transp(xns.pop(m), m)
                    ps = proj(w_q, m, pp, bq_sb, 0)
                    if m == 0:
                        w_k = stream_w(wk_d)   # lands during the q loop
                    if m >= 3:
                        transp(qrs.pop(m - 3), m - 3, qT)
                    qrs[m] = rms_rope(ps, m)
                    if m + 2 < MT:
                        xns[m + 2] = ln_math(m + 2)
                for mm in (MT - 3, MT - 2, MT - 1):
                    transp(qrs.pop(mm), mm, qT)

                # ---- K loop ----
                w_v = None
                for m in range(MT):
                    ps = proj(w_k, m, pp, bkv_sb, 0)
                    if m == 0:
                        w_v = stream_w(wv_d)
                    if m >= 3:
                        transp(qrs.pop(m - 3), m - 3, kT)
                    qrs[m] = rms_rope(ps, m)
                def v_proj(m):
                    ps = proj(w_v, m, pp, bkv_sb, INNER)
                    for nh in range(2):
                        nc.vector.tensor_copy(
                            v_sb[:, m, nh * 8:(nh + 1) * 8, 0:DH],
                            ps[:, nh * 512:(nh + 1) * 512].rearrange(
                                "p (h d) -> p h d", h=8),
                        )

                def qk_exp_step(f, j, dpool):
                    """QK + exp for key tile j of head pair f."""
                    dots_ts = []
                    for hb in range(2):
                        dots = dpool.tile([128, N], F32, tag="dots", bufs=2,
                                          name=f"dots{hb}")
                        dots_ts.append(dots)
                    # Key-half split: each matmul covers 64 keys so the two
                    # heads' matmuls have disjoint PE row groups AND disjoint
                    # output partitions — they execute concurrently as tiles.
                    for qh in range(2):
                        qsl = slice(qh * 512, (qh + 1) * 512)
                        for kh in range(2):
                            for hb in range(2):
                                pb_ = hb * 64
                                oh = (kh + hb) % 2   # A-lo||B-hi, A-hi||B-lo
                                ksl = slice(j * 128 + oh * 64,
                                            j * 128 + (oh + 1) * 64)
                                nc.tensor.matmul(
                                    dots_ts[hb][oh * 64:(oh + 1) * 64, qsl],
                                    kT[pb_:pb_ + 64, f, ksl],
                                    qT[pb_:pb_ + 64, f, qsl],
                                    start=True, stop=True,
                                )
                    es = []
                    for hb in range(2):
                        e_t = ep.tile([128, N], BF16, tag="E", bufs=26)
                        nc.scalar.activation(
                            e_t[:], dots_ts[hb][:],
                            mybir.ActivationFunctionType.Exp,
                        )
                        es.append(e_t)
                    return es

                # ---- k tail + V loop interleaved with pair-0 QK/exp ----
                # v_proj(0) slots in before the last k transpose (which waits
                # on the GPS chain); tp then closes so dp0 gets its banks.
                transp(qrs.pop(MT - 3), MT - 3, kT)
                transp(qrs.pop(MT - 2), MT - 2, kT)
                v_proj(0)
                transp(qrs.pop(MT - 1), MT - 1, kT)
                tp_cm.__exit__(None, None, None)
                dp0_cm = tc.tile_pool(name="dp0", bufs=2, space="PSUM")
                dp0 = dp0_cm.__enter__()
                w_o = None
                sched = {1: [0], 2: [1], 3: [2], 4: [3], 5: [4], 6: [5], 7: [6, 7]}
                for m in range(1, MT):
                    v_proj(m)
                    if m == 1:
                        w_o = stream_w(wo_d)
                    for j in sched.get(m, []):
                        es_pair0[j] = qk_exp_step(0, j, dp0)
                dp0_cm.__exit__(None, None, None)
                pp_cm.__exit__(None, None, None)

            # ---------------- Phase C: attention ----------------
            outT = xnT  # xnT is dead; reuse as normalized attention output
            with (
                tc.tile_pool(name="dp", bufs=2, space="PSUM") as dp,
                tc.tile_pool(name="op", bufs=4, space="PSUM") as op,
            ):
                def av_step(f, j, es_all, oas, hbs=(0, 1)):
                    for hb in hbs:
                        h = 2 * f + hb
                        for qh in range(2):
                            nc.tensor.matmul(
                                oas[hb][qh], v_sb[:, j, h, :],
                                es_all[j][hb][:, qh * 512:(qh + 1) * 512],
                                start=(j == 0), stop=(j == MT - 1),
                            )

                def av_alloc():
                    oas = []
                    for hb in range(2):
                        row = []
                        for qh in range(2):
                            oa = op.tile([DH + 1, 512], F32, tag="oa",
                                         bufs=4, name=f"oa{hb}{qh}")
                            row.append(oa)
                        oas.append(row)
                    return oas

                def av_finish(f, oas):
                    """Evict accumulators, softmax-normalize via a DRAM-bounce
                    partition broadcast, write fp16 output into outT."""
                    dpair = cpool.tile([2, N], F32, tag="dpair", bufs=1)
                    for hb in range(2):
                        pb_ = hb * 64
                        for qh in range(2):
                            nc.vector.tensor_copy(
                                qT[pb_:pb_ + 64, f,
                                   qh * 512:(qh + 1) * 512].bitcast(BF16),
                                oas[hb][qh][0:DH, :],
                            )
                            drow = cpool.tile([1, 512], F32, tag="drow", bufs=2)
                            nc.vector.tensor_copy(drow[:], oas[hb][qh][DH:DH + 1, :])
                            nc.sync.dma_start(
                                dpair[hb:hb + 1, qh * 512:(qh + 1) * 512],
                                drow[:],
                            )
                    rd2 = cpool.tile([2, N], F32, tag="rd2", bufs=1)
                    nc.vector.reciprocal_approx_fast(rd2[:], dpair[:])
                    nc.sync.dma_start(rd_dram[f, :, :], rd2[:])
                    rb = cpool.tile([128, N], F32, tag="rb", bufs=1)
                    for hb in range(2):
                        nc.sync.dma_start(
                            rb[hb * 64:(hb + 1) * 64, :],
                            bass.AP(
                                tensor=rd_dram,
                                offset=rd_dram[f, hb, :].offset,
                                ap=[[0, 64], [1, N]],
                            ),
                        )
                    nc.gpsimd.tensor_mul(outT[:, f, :], qT[:, f, :].bitcast(BF16), rb[:])

                es_prev = es_pair0
                for f in range(1, FT):
                    es_cur = {}
                    oas_cur = av_alloc()
                    for j in range(MT):
                        av_step(f - 1, j, es_prev, oas_cur)
                        es_cur[j] = qk_exp_step(f, j, dp)
                    av_finish(f - 1, oas_cur)
                    es_prev = es_cur
                oas_last = av_alloc()
                for j in range(MT):
                    av_step(FT - 1, j, es_prev, oas_last)
                av_finish(FT - 1, oas_last)

            # ---------------- Phase D: Wo projection ----------------
            with (
                tc.tile_pool(name="fin", bufs=2) as fin,
                tc.tile_pool(name="fp", bufs=3, space="PSUM") as fp,
            ):
                for m in range(MT):
                    ps = fp.tile([128, DIM], F32, tag="fp", bufs=3)
                    for f in list(range(FT - 1)) + [FT - 1]:
                        for nh in range(2):
                            sl = slice(nh * 512, (nh + 1) * 512)
                            nc.tensor.matmul(
                                ps[:, sl],
                                outT[:, f, m * 128:(m + 1) * 128],
                                w_o[:, f, nh * 512:(nh + 1) * 512],
                                start=(f == 0), stop=(f == FT - 1),
                            )
                    fs = fin.tile([128, DIM], F32, tag="fs", bufs=2)
                    for nh in range(2):
                        sl = slice(nh * 512, (nh + 1) * 512)
                        if nh == 0:
                            nc.scalar.copy(fs[:, sl], ps[:, sl])
                        else:
                            nc.vector.tensor_copy(fs[:, sl], ps[:, sl])
                        nc.sync.dma_start(
                            out_d[m * 128:(m + 1) * 128, sl], fs[:, sl])

            cp_cm.__exit__(None, None, None)
            ep_cm.__exit__(None, None, None)

    nc.compile()
    return nc


import numpy as np
from concourse.bass_utils import run_bass_kernel_spmd

_NC_CACHE = {}


def _get_nc(has_bias: bool):
    if has_bias not in _NC_CACHE:
        _NC_CACHE[has_bias] = build_nc(has_bias)
    return _NC_CACHE[has_bias]


def _pmajor(w):
    """[DIM, C] -> [128, KT*C] partition-major relayout (contiguous DMA)."""
    d, c = w.shape
    kt = d // 128
    return np.ascontiguousarray(
        w.reshape(kt, 128, c).transpose(1, 0, 2).reshape(128, kt * c))


def host_prepare(x, mask, h_idx, w_idx, gamma_ln, beta_ln, q_gamma, k_gamma,
                 Wq, Wkv, Wo):
    x = np.asarray(x, np.float32)
    mask = np.asarray(mask)
    assert mask.all(), "kernel assumes all-True mask"
    assert np.allclose(np.asarray(q_gamma), 1.0), "kernel assumes q_gamma == 1"
    assert np.allclose(np.asarray(k_gamma), 1.0), "kernel assumes k_gamma == 1"

    gamma_ln = np.asarray(gamma_ln, np.float32)
    beta_ln = np.asarray(beta_ln, np.float32)
    Wq = np.asarray(Wq, np.float32)
    Wkv = np.asarray(Wkv, np.float32)
    Wo = np.asarray(Wo, np.float32)

    wq_f = _pmajor((gamma_ln[:, None] * Wq).astype(np.float16))
    wkv_g = (gamma_ln[:, None] * Wkv).astype(np.float16)
    wk_f = _pmajor(wkv_g[:, :INNER])
    wv_f = _pmajor(wkv_g[:, INNER:])
    wo_f = _pmajor(Wo.astype(np.float16))
    bq = (beta_ln @ Wq)[None, :].astype(np.float32)
    bkv = (beta_ln @ Wkv)[None, :].astype(np.float32)
    has_bias = bool(np.abs(bq).max() > 0 or np.abs(bkv).max() > 0)

    # RoPE tables; sqrt(DH)=8 and rotate-half signs folded in.
    h_idx = np.asarray(h_idx, np.float32)
    w_idx = np.asarray(w_idx, np.float32)
    dq = DH // 4
    inv_freq = 1.0 / (10000.0 ** (np.arange(dq, dtype=np.float32) / dq))
    th = h_idx[..., None] * inv_freq
    tw = w_idx[..., None] * inv_freq
    cos_t = (np.concatenate([np.cos(th), np.cos(th), np.cos(tw), np.cos(tw)], -1)
             * np.sqrt(np.float32(DH))).astype(np.float16)
    sin_full = (np.concatenate([np.sin(th), np.sin(th), np.sin(tw), np.sin(tw)], -1)
                * np.sqrt(np.float32(DH)))
    sign = np.tile(np.concatenate(
        [-np.ones(dq, np.float32), np.ones(dq, np.float32)]), 2)
    sin_t = (sin_full * sign).astype(np.float16)

    def trig_pmajor(t):
        # [N, DH] -> [128, MT*DH] partition-major
        return np.ascontiguousarray(
            t.reshape(MT, 128, DH).transpose(1, 0, 2).reshape(128, MT * DH))

    in_maps = []
    for b in range(B):
        m = {
            "x": np.ascontiguousarray(x[b]),
            "ident16": np.eye(128, dtype=np.float16),
            "wq": wq_f,
            "wk": wk_f,
            "wv": wv_f,
            "wo": wo_f,
            "cos_t": trig_pmajor(cos_t[b]),
            "sin_t": trig_pmajor(sin_t[b]),
        }
        if has_bias:
            m["bq"] = bq.astype(np.float16)
            m["bkv"] = bkv.astype(np.float16)
        in_maps.append(m)
    return in_maps, has_bias


def run(trace=False, **inputs):
    in_maps, has_bias = host_prepare(**inputs)
    nc = _get_nc(has_bias)
    res = run_bass_kernel_spmd(nc, in_maps, core_ids=list(range(B)), trace=trace)
    out = np.stack([res.results[c]["out"] for c in range(B)], axis=0)
    return out.astype(np.float32), res


def kernel(**inputs):
    out, _ = run(trace=False, **inputs)
    return out


if __name__ == "__main__":
    build_nc(False)
    print("build ok")
